# revision 30
# baseline (speedup 1.0000x reference)
"""Trainium2 Bass kernel for causal multi-head attention (v3, interleaved).

Problem: B=2, T=4096, D=768, H=12 heads, d_k=64, causal mask.
Sharding: 8 cores = 2 batches x 4 head-groups (3 heads each).
Each core computes its batch's qkv projection (its heads only), flash-style
attention with transposed scores (S^T = k q^T, so softmax statistics land in
the matmul-friendly layout with no P-transposes), and a partial output
projection. Host sums the 4 head-group partials per batch and adds the
folded bias constant (v-bias @ W_out + b_out). The k-bias is dropped
entirely (softmax is invariant to per-query score shifts).

v3: the qkv-projection and out-projection matmuls are interleaved into the
attention j-block loop as PE "filler" quanta.  In v2 the attention pipeline
alternated PE<->ACT with ~200-600ns PE idle per j-block, which kept the PE
HAM throttle oscillating (338us at half clock).  Filler keeps the PE stream
dense: chunk ich's attention hosts proj(ich+1) and outproj(ich-1) quanta,
placed between the score matmuls/exp and the context matmuls of each
j-block, exactly where the PE would otherwise wait for the exp.

All matmul operands bf16 (host-cast; tolerance is 2e-2, bf16 noise ~3e-3).
x^T comes from xbar DMA-transpose.  exp/score/context skip fully-masked
query columns of diagonal j-blocks.

Self-contained: hardcodes all shapes; only imports the concourse runtime.
"""

import sys

sys.path.insert(0, "/opt/trn_rl_repo")

from contextlib import ExitStack

import ml_dtypes
import numpy as np

import concourse.bass as bass  # noqa: F401
import concourse.mybir as mybir
import concourse.tile as tile
from concourse import bacc
from concourse.bass_utils import run_bass_kernel_spmd

F32 = mybir.dt.float32
BF16 = mybir.dt.bfloat16
NPBF16 = ml_dtypes.bfloat16

B, T, D = 2, 4096, 768
H, DK = 12, 64
HPC = 3          # heads per core
N_CORES = 8
ICH_W = 512      # i-chunk width (queries per outer step)
JB_W = 128       # j-block width (keys per matmul)
KT = D // 128    # 6 contraction tiles for the projections
EXP = mybir.ActivationFunctionType.Exp
SCALE = 1.0 / np.sqrt(DK)


def build_program(t=T):
    """Build the SPMD Bass program for one core (all cores identical)."""
    n_ich = t // ICH_W

    nc = bacc.Bacc("TRN2", target_bir_lowering=False, debug=False,
                   num_devices=N_CORES)

    # x arrives pre-transposed from the host: [D, t] bf16.  This removes all
    # xbar DMA-transposes (48 x ~1.3us of serialized descriptor-gen on the
    # sync queue) - each chunk's x^T loads as ONE plain 3D-AP DMA.
    x_d = nc.dram_tensor("x", [D, t], BF16, kind="ExternalInput").ap()
    # qk projection weights, 4 chunks of 128 output channels:
    # ch0=[q0|q1] ch1=[k0|k1] ch2=[q2|k2] ch3=[k2|q2]
    wqk_d = nc.dram_tensor("wqk", [D, 512], BF16, kind="ExternalInput").ap()
    bqk_d = nc.dram_tensor("bqk", [512], F32, kind="ExternalInput").ap()
    wv_d = nc.dram_tensor("wv", [D, HPC * DK], BF16, kind="ExternalInput").ap()
    wout_d = nc.dram_tensor("wout", [HPC * DK, D], BF16,
                            kind="ExternalInput").ap()
    out_d = nc.dram_tensor("out", [t, D], F32, kind="ExternalOutput").ap()

    with tile.TileContext(nc) as tc, ExitStack() as top:
        consts = top.enter_context(tc.tile_pool(name="consts", bufs=1))
        persist = top.enter_context(tc.tile_pool(name="persist", bufs=1))
        xtp = top.enter_context(tc.tile_pool(name="xt", bufs=2))
        ptp = top.enter_context(tc.tile_pool(name="pt", bufs=3))
        ctxp = top.enter_context(tc.tile_pool(name="ctx", bufs=12))
        smp = top.enter_context(tc.tile_pool(name="sm", bufs=4))
        outp = top.enter_context(tc.tile_pool(name="outsb", bufs=2))
        # PSUM: st 2 banks x2 + cps 1 bank x2 + pj 1 bank + op 1 bank = 8
        stp = top.enter_context(tc.tile_pool(name="stp", bufs=2, space="PSUM"))
        cpp = top.enter_context(tc.tile_pool(name="cpp", bufs=2, space="PSUM"))
        pjp = top.enter_context(tc.tile_pool(name="pjp", bufs=1, space="PSUM"))
        opp = top.enter_context(tc.tile_pool(name="opp", bufs=1, space="PSUM"))

        # q^T / k^T per chunk: [128, 4, t] bf16
        qk_sb = persist.tile([128, 4, t], BF16)
        # v (natural layout) + ones column: [128, n_tch, HPC, 65] bf16
        vaug = persist.tile([128, t // 128, HPC, DK + 1], BF16)

        xts = {}
        ctxn_store = {}

        def issue_xt(ich):
            """Prefetch chunk ich's x^T via xbar DMA-transpose.

            The first chunks split their transposes across BOTH HWDGE
            queues (sync + scalar): at startup the single sync queue
            serializes at ~2.7us per transpose (issue + transfer, ~2 in
            flight) which starves the first projections; the ACT queue is
            idle then.  Later chunks keep everything on sync so the
            descriptor generation (~1.3us each) never displaces exp work
            on the ACT sequencer."""
            if ich >= n_ich:
                return
            i0 = ich * ICH_W
            xt = xtp.tile([128, KT, ICH_W], BF16, tag="xt", name=f"xt{ich}")
            nc.sync.dma_start(
                out=xt,
                in_=x_d.rearrange("(kt p) t -> p kt t", p=128)[:, :,
                                                              i0:i0 + ICH_W])
            xts[ich] = xt

        IDENT = mybir.ActivationFunctionType.Identity

        def proj_quanta(ich):
            """Closures emitting chunk ich's projections.

            Each qk/v group is ONE quantum of 6 matmuls plus a separate
            epilogue quantum on the ACT engine (Identity with per-partition
            bias) so the pj PSUM ring slot is held for only one filler point
            and its release never queues behind long DVE ops."""
            i0 = ich * ICH_W
            qlist = []

            def mk_qk(ch):
                st_ = {}

                def q0():
                    ps = pjp.tile([128, ICH_W], F32, tag="pj", space="PSUM",
                                  name=f"qps{ich}_{ch}")
                    st_["ps"] = ps
                    for kt in range(KT):
                        nc.tensor.matmul(
                            ps, lhsT=wqk_sb[:, kt, ch * 128:(ch + 1) * 128],
                            rhs=xts[ich][:, kt, :],
                            start=(kt == 0), stop=(kt == KT - 1))

                def q1():
                    nc.scalar.activation(
                        qk_sb[:, ch, i0:i0 + ICH_W], st_["ps"], IDENT,
                        bias=bqk_sb[:, ch:ch + 1], scale=1.0)

                return [q0, q1]

            def mk_v(tl):
                st_ = {}
                tch = ich * (ICH_W // 128) + tl

                def q0():
                    ps = pjp.tile([128, ICH_W], F32, tag="pj", space="PSUM",
                                  name=f"vps{ich}_{tl}")
                    st_["ps"] = ps
                    for kt in range(KT):
                        nc.tensor.matmul(
                            ps[:, 0:HPC * DK],
                            lhsT=xts[ich][:, kt, tl * 128:(tl + 1) * 128],
                            rhs=wv_sb[:, kt, :],
                            start=(kt == 0), stop=(kt == KT - 1))

                def q1():
                    nc.scalar.activation(
                        vaug[:, tch, :, 0:DK],
                        st_["ps"][:, 0:HPC * DK].rearrange(
                            "p (h d) -> p h d", h=HPC),
                        IDENT, bias=0.0, scale=1.0)

                return [q0, q1]

            for ch in range(4):
                qlist += mk_qk(ch)
            for tl in range(ICH_W // 128):
                qlist += mk_v(tl)
            qlist.append(lambda: issue_xt(ich + 1))
            return qlist

        def outproj_quanta(ich):
            """Closures emitting chunk ich's out-projection in quanta."""
            i0 = ich * ICH_W
            qlist = []

            def mk(tsub):
                st_ = {}

                def q0():
                    # normalize muls for this chunk are deferred closures;
                    # make sure they have landed before reading ctxn.
                    while len(ctxn_store.get(ich, {})) < HPC:
                        deferred.pop(0)()
                    ctxn = ctxn_store[ich]
                    st_["osb"] = outp.tile([128, D], F32, tag="osb",
                                           name=f"osb{ich}_{tsub}")
                    o1 = opp.tile([128, 512], F32, tag="op", space="PSUM",
                                  name=f"op1_{ich}_{tsub}")
                    st_["o1"] = o1
                    for h in range(HPC):
                        nc.tensor.matmul(
                            o1, lhsT=ctxn[h][:, tsub * 128:(tsub + 1) * 128],
                            rhs=wout_sb[:, h, 0:512],
                            start=(h == 0), stop=(h == HPC - 1))

                def q1():
                    nc.vector.tensor_copy(st_["osb"][:, 0:512], st_["o1"])

                def q2():
                    ctxn = ctxn_store[ich]
                    o2 = opp.tile([128, 512], F32, tag="op", space="PSUM",
                                  name=f"op2_{ich}_{tsub}")
                    st_["o2"] = o2
                    for h in range(HPC):
                        nc.tensor.matmul(
                            o2[:, 0:256],
                            lhsT=ctxn[h][:, tsub * 128:(tsub + 1) * 128],
                            rhs=wout_sb[:, h, 512:D],
                            start=(h == 0), stop=(h == HPC - 1))

                def q3():
                    nc.vector.tensor_copy(st_["osb"][:, 512:D],
                                          st_["o2"][:, 0:256])
                    nc.sync.dma_start(
                        out=out_d[i0 + tsub * 128:i0 + (tsub + 1) * 128, :],
                        in_=st_["osb"])

                return [q0, q1, q2, q3]

            for tsub in range(ICH_W // 128):
                qlist += mk(tsub)
            return qlist

        deferred = []   # small DVE/gpsimd closures spread across filler points

        def normalize(ich, h, cps):
            # copy PSUM->SBUF immediately so the cps ring slot frees in
            # ~0.7us.  The division itself runs on GPSIMD (broadcast the raw
            # denominator, then tensor/tensor divide): a DVE reciprocal is
            # ~6.5 cyc/elem and its ~10us/chunk mass in the FIFO was stalling
            # the small epilogue ops that gate PE matmuls.
            ctxf = smp.tile([65, ICH_W], F32, tag="ctxf", bufs=8,
                            name=f"cf{ich}{h}")
            nc.vector.tensor_copy(ctxf, cps)
            recip = smp.tile([1, ICH_W], F32, tag="recip", bufs=8,
                             name=f"rc{ich}{h}")

            def mk_recip(c):
                return lambda: nc.vector.reciprocal(
                    recip[:, c * 128:(c + 1) * 128],
                    ctxf[64:65, c * 128:(c + 1) * 128])

            def bcast():
                rb = smp.tile([64, ICH_W], F32, tag="rb", bufs=6,
                              name=f"rb{ich}{h}")
                nc.gpsimd.partition_broadcast(rb, recip)
                st_["rb"] = rb

            def mul():
                cn = ctxp.tile([64, ICH_W], BF16, tag="ctxn",
                               name=f"cn{ich}{h}")
                nc.vector.tensor_mul(cn, ctxf[0:64, :], st_["rb"])
                ctxn_store.setdefault(ich, {})[h] = cn

            st_ = {}
            deferred.extend([mk_recip(c) for c in range(ICH_W // 128)])
            deferred.append(bcast)
            deferred.append(mul)

        def attn_chunk(ich, fill):
            i0 = ich * ICH_W
            njb = (i0 + ICH_W) // JB_W
            points = njb + njb // 2
            fk = [0]

            def filler():
                n = 1 + (len(deferred) > 6) + (len(deferred) > 12)
                for _ in range(min(n, len(deferred))):
                    deferred.pop(0)()
                a = fk[0] * len(fill) // points
                b = (fk[0] + 1) * len(fill) // points
                for f in fill[a:b]:
                    f()
                fk[0] += 1

            # ---- pass A: heads 0 and 1, row-group paired ----
            cps0 = cpp.tile([65, ICH_W], F32, tag="cps", space="PSUM",
                            name=f"cps0_{ich}")
            cps1 = cpp.tile([65, ICH_W], F32, tag="cps", space="PSUM",
                            name=f"cps1_{ich}")
            for jb in range(njb):
                j0 = jb * JB_W
                s = jb - (njb - 4)          # diag position if >= 0
                c0 = 128 * s if s > 0 else 0
                st = stp.tile([128, 2, ICH_W], F32, tag="st", space="PSUM",
                              name=f"stA{ich}_{jb}")
                nc.tensor.matmul(
                    st[:, 0, c0:],
                    lhsT=qk_sb[0:64, 1, j0:j0 + JB_W],
                    rhs=qk_sb[0:64, 0, i0 + c0:i0 + ICH_W],
                    start=True, stop=True)
                nc.tensor.matmul(
                    st[:, 1, c0:],
                    lhsT=qk_sb[64:128, 1, j0:j0 + JB_W],
                    rhs=qk_sb[64:128, 0, i0 + c0:i0 + ICH_W],
                    start=True, stop=True)
                pt = ptp.tile([128, 2, ICH_W], BF16, tag="pt",
                              name=f"ptA{ich}_{jb}")
                nc.scalar.activation(pt[:, :, c0:], st[:, :, c0:], EXP,
                                     bias=0.0, scale=SCALE)
                if s >= 0:
                    for hh in range(2):
                        nc.gpsimd.affine_select(
                            out=pt[:, hh, c0:c0 + 128],
                            in_=pt[:, hh, c0:c0 + 128],
                            compare_op=mybir.AluOpType.is_ge,
                            fill=0.0, base=0,
                            pattern=[[1, 128]], channel_multiplier=-1)
                filler()
                nc.tensor.matmul(
                    cps0[:, c0:], lhsT=vaug[:, jb, 0, :], rhs=pt[:, 0, c0:],
                    start=(jb == 0), stop=(jb == njb - 1))
                nc.tensor.matmul(
                    cps1[:, c0:], lhsT=vaug[:, jb, 1, :], rhs=pt[:, 1, c0:],
                    start=(jb == 0), stop=(jb == njb - 1))

            normalize(ich, 0, cps0)
            normalize(ich, 1, cps1)

            # ---- pass B: head 2, alternating row groups ----
            cps2 = cpp.tile([65, ICH_W], F32, tag="cps", space="PSUM",
                            name=f"cps2_{ich}")
            for grp in range(njb // 2):
                st = stp.tile([128, 2, ICH_W], F32, tag="st", space="PSUM",
                              name=f"stB{ich}_{grp}")
                pt = ptp.tile([128, 2, ICH_W], BF16, tag="pt",
                              name=f"ptB{ich}_{grp}")
                c0s = []
                for jj in range(2):
                    jb = grp * 2 + jj
                    j0 = jb * JB_W
                    s = jb - (njb - 4)
                    c0 = 128 * s if s > 0 else 0
                    c0s.append((s, c0))
                    if jb % 2 == 0:
                        lhsT = qk_sb[0:64, 3, j0:j0 + JB_W]
                        rhs = qk_sb[0:64, 2, i0 + c0:i0 + ICH_W]
                    else:
                        lhsT = qk_sb[64:128, 2, j0:j0 + JB_W]
                        rhs = qk_sb[64:128, 3, i0 + c0:i0 + ICH_W]
                    nc.tensor.matmul(st[:, jj, c0:], lhsT=lhsT, rhs=rhs,
                                     start=True, stop=True)
                if c0s[0][0] < 0 and c0s[1][0] < 0:
                    # off-diagonal pair: one fused exp
                    nc.scalar.activation(pt, st, EXP, bias=0.0, scale=SCALE)
                else:
                    for jj in range(2):
                        c0 = c0s[jj][1]
                        nc.scalar.activation(pt[:, jj, c0:], st[:, jj, c0:],
                                             EXP, bias=0.0, scale=SCALE)
                for jj in range(2):
                    s, c0 = c0s[jj]
                    if s >= 0:
                        nc.gpsimd.affine_select(
                            out=pt[:, jj, c0:c0 + 128],
                            in_=pt[:, jj, c0:c0 + 128],
                            compare_op=mybir.AluOpType.is_ge,
                            fill=0.0, base=0,
                            pattern=[[1, 128]], channel_multiplier=-1)
                filler()
                for jj in range(2):
                    jb = grp * 2 + jj
                    c0 = c0s[jj][1]
                    nc.tensor.matmul(
                        cps2[:, c0:], lhsT=vaug[:, jb, 2, :],
                        rhs=pt[:, jj, c0:],
                        start=(jb == 0), stop=(jb == njb - 1))

            normalize(ich, 2, cps2)
            # drain any leftover filler (rounding)
            for f in fill[fk[0] * len(fill) // points:]:
                f()

        # ---------------- main schedule ----------------
        # Startup: interleave weight loads with chunk 0's x^T transposes so
        # the first projection matmuls start as soon as their operands land.
        wqk_r = wqk_d.rearrange("(kt p) c -> p kt c", p=128)
        wqk_sb = consts.tile([128, KT, 512], BF16)
        bqk_sb = consts.tile([128, 4], F32)
        wv_sb = consts.tile([128, KT, HPC * DK], BF16)
        wout_sb = consts.tile([64, HPC, D], BF16)
        nc.sync.dma_start(out=wqk_sb[:, 0:2, :], in_=wqk_r[:, 0:2, :])
        nc.sync.dma_start(out=bqk_sb,
                          in_=bqk_d.rearrange("(ch p) -> p ch", p=128))
        issue_xt(0)
        xt0 = xts[0]
        nc.sync.dma_start(out=wqk_sb[:, 2:KT, :], in_=wqk_r[:, 2:KT, :])
        nc.sync.dma_start(out=wv_sb,
                          in_=wv_d.rearrange("(kt p) c -> p kt c", p=128))
        nc.sync.dma_start(out=wout_sb,
                          in_=wout_d.rearrange("(h p) m -> p h m", p=64))
        # softmax-denominator ones column, written once
        nc.vector.memset(vaug[:, :, :, DK:DK + 1], 1.0)
        # proj(0) runs kt-major over PAIRS of output chunks (using the idle
        # pj+op rings) so each matmul waits only on its own x^T transpose
        # instead of the whole 6-DMA chain.
        for ch0 in (0, 2):
            psa = pjp.tile([128, ICH_W], F32, tag="pj", space="PSUM",
                           name=f"qps0_{ch0}")
            psb = opp.tile([128, ICH_W], F32, tag="op", space="PSUM",
                           name=f"qps0_{ch0 + 1}")
            for kt in range(KT):
                for ps, ch in ((psa, ch0), (psb, ch0 + 1)):
                    nc.tensor.matmul(
                        ps, lhsT=wqk_sb[:, kt, ch * 128:(ch + 1) * 128],
                        rhs=xt0[:, kt, :],
                        start=(kt == 0), stop=(kt == KT - 1))
            for ps, ch in ((psa, ch0), (psb, ch0 + 1)):
                nc.scalar.activation(
                    qk_sb[:, ch, 0:ICH_W], ps,
                    mybir.ActivationFunctionType.Identity,
                    bias=bqk_sb[:, ch:ch + 1], scale=1.0)
        for tl in range(ICH_W // 128):
            ps = pjp.tile([128, ICH_W], F32, tag="pj", space="PSUM",
                          name=f"vps0_{tl}")
            for kt in range(KT):
                nc.tensor.matmul(
                    ps[:, 0:HPC * DK],
                    lhsT=xt0[:, kt, tl * 128:(tl + 1) * 128],
                    rhs=wv_sb[:, kt, :],
                    start=(kt == 0), stop=(kt == KT - 1))
            nc.scalar.activation(
                vaug[:, tl, :, 0:DK],
                ps[:, 0:HPC * DK].rearrange("p (h d) -> p h d", h=HPC),
                mybir.ActivationFunctionType.Identity, bias=0.0, scale=1.0)
        issue_xt(1)
        # outproj(ich) is hosted three chunks later: the ctxn normalize chain
        # gets two full chunks of slack (its deferred DVE ops never gate the
        # outproj matmuls) and the last, ACT-bound chunk receives ~20us of
        # real PE filler, which also keeps the HAM clock warm there.
        hosts = {src: min(src + 3, n_ich - 1) for src in range(n_ich - 1)}
        for ich in range(n_ich):
            fill = []
            if ich + 1 < n_ich:
                fill += proj_quanta(ich + 1)
            for src, host in hosts.items():
                if host == ich:
                    fill += outproj_quanta(src)
            attn_chunk(ich, fill)
        while deferred:
            deferred.pop(0)()
        for q in outproj_quanta(n_ich - 1):
            q()

    nc.compile()
    return nc


def make_core_inputs(x_b, W_qkv, b_qkv, W_out, hg):
    """Host-side weight slicing/permutation for one head-group hg (0..3)."""
    heads = [hg * HPC + i for i in range(HPC)]
    # W_qkv last-dim layout: c = h*192 + s*64 + d  (s: 0=q 1=k 2=v)
    def cols(h, s):
        return slice(h * 192 + s * 64, h * 192 + s * 64 + 64)

    q = [np.asarray(W_qkv[:, cols(h, 0)]) for h in heads]
    k = [np.asarray(W_qkv[:, cols(h, 1)]) for h in heads]
    v = [np.asarray(W_qkv[:, cols(h, 2)]) for h in heads]
    bq = [np.asarray(b_qkv[cols(h, 0)]) for h in heads]

    wqk = np.concatenate([q[0], q[1], k[0], k[1], q[2], k[2], k[2], q[2]],
                         axis=1)
    z = np.zeros(64, np.float32)
    bqk = np.concatenate([bq[0], bq[1], z, z, bq[2], z, z, bq[2]]).astype(
        np.float32)
    wv = np.concatenate(v, axis=1)
    wout = np.concatenate(
        [np.asarray(W_out[h * DK:(h + 1) * DK, :]) for h in heads], axis=0)
    return {
        "x": np.ascontiguousarray(np.asarray(x_b).astype(NPBF16).T),
        "wqk": np.ascontiguousarray(wqk.astype(NPBF16)),
        "bqk": np.ascontiguousarray(bqk),
        "wv": np.ascontiguousarray(wv.astype(NPBF16)),
        "wout": np.ascontiguousarray(wout.astype(NPBF16)),
    }


_CACHE = {}


def _get_program(t=T):
    if t not in _CACHE:
        _CACHE[t] = build_program(t)
    return _CACHE[t]


def run_cores(inputs, t=T, trace=False):
    nc = _get_program(t)
    x = np.asarray(inputs["x"], np.float32)
    in_maps = []
    for core in range(N_CORES):
        b, hg = core // 4, core % 4
        in_maps.append(make_core_inputs(x[b], inputs["W_qkv"],
                                        inputs["b_qkv"], inputs["W_out"], hg))
    res = run_bass_kernel_spmd(nc, in_maps, list(range(N_CORES)), trace=trace)
    return res


def gather(inputs, results):
    b_qkv = np.asarray(inputs["b_qkv"], np.float32)
    W_out = np.asarray(inputs["W_out"], np.float32)
    b_out = np.asarray(inputs["b_out"], np.float32)
    bv = np.concatenate([b_qkv[h * 192 + 128:h * 192 + 192] for h in range(H)])
    fold = bv @ W_out + b_out                      # [D]
    t = results[0]["out"].shape[0]
    out = np.zeros((B, t, D), np.float32)
    for core in range(N_CORES):
        out[core // 4] += results[core]["out"]
    out += fold[None, None, :]
    return out


def kernel(**inputs):
    res = run_cores(inputs)
    return gather(inputs, res.results)


if __name__ == "__main__":
    # smoke test with random data
    rng = np.random.default_rng(0)
    inputs = {
        "x": rng.standard_normal((B, T, D), dtype=np.float32),
        "mask": np.triu(np.ones((T, T), dtype=bool), k=1),
        "W_qkv": (rng.standard_normal((D, 3 * D), dtype=np.float32)
                  / np.sqrt(D)),
        "b_qkv": rng.standard_normal(3 * D).astype(np.float32) * 0.02,
        "W_out": (rng.standard_normal((D, D), dtype=np.float32)
                  / np.sqrt(D)),
        "b_out": rng.standard_normal(D).astype(np.float32) * 0.02,
    }
    out = kernel(**inputs)
    print(out.shape, out.dtype)


# revision 34
# speedup vs baseline: 1.1927x; 1.1927x over previous
"""Trainium2 Bass kernel for causal multi-head attention (v3, interleaved).

Problem: B=2, T=4096, D=768, H=12 heads, d_k=64, causal mask.
Sharding: 8 cores = 2 batches x 4 head-groups (3 heads each).
Each core computes its batch's qkv projection (its heads only), flash-style
attention with transposed scores (S^T = k q^T, so softmax statistics land in
the matmul-friendly layout with no P-transposes), and a partial output
projection. Host sums the 4 head-group partials per batch and adds the
folded bias constant (v-bias @ W_out + b_out). The k-bias is dropped
entirely (softmax is invariant to per-query score shifts).

v3: the qkv-projection and out-projection matmuls are interleaved into the
attention j-block loop as PE "filler" quanta.  In v2 the attention pipeline
alternated PE<->ACT with ~200-600ns PE idle per j-block, which kept the PE
HAM throttle oscillating (338us at half clock).  Filler keeps the PE stream
dense: chunk ich's attention hosts proj(ich+1) and outproj(ich-1) quanta,
placed between the score matmuls/exp and the context matmuls of each
j-block, exactly where the PE would otherwise wait for the exp.

All matmul operands bf16 (host-cast; tolerance is 2e-2, bf16 noise ~3e-3).
x^T comes from xbar DMA-transpose.  exp/score/context skip fully-masked
query columns of diagonal j-blocks.

Self-contained: hardcodes all shapes; only imports the concourse runtime.
"""

import sys

sys.path.insert(0, "/opt/trn_rl_repo")

from contextlib import ExitStack

import ml_dtypes
import numpy as np

import concourse.bass as bass  # noqa: F401
import concourse.mybir as mybir
import concourse.tile as tile
from concourse import bacc
from concourse.bass_utils import run_bass_kernel_spmd

F32 = mybir.dt.float32
BF16 = mybir.dt.bfloat16
NPBF16 = ml_dtypes.bfloat16

B, T, D = 2, 4096, 768
H, DK = 12, 64
HPC = 3          # heads per core
N_CORES = 8
ICH_W = 512      # i-chunk width (queries per outer step)
JB_W = 128       # j-block width (keys per matmul)
KT = D // 128    # 6 contraction tiles for the projections
EXP = mybir.ActivationFunctionType.Exp
SCALE = 1.0 / np.sqrt(DK)


def build_program(t=T):
    """Build the SPMD Bass program for one core (all cores identical)."""
    n_ich = t // ICH_W

    nc = bacc.Bacc("TRN2", target_bir_lowering=False, debug=False,
                   num_devices=N_CORES)

    # x arrives pre-transposed AND chunk-major from the host:
    # x[p, ich, kt, tl] = x^T[kt*128+p, ich*512+tl].  This removes all xbar
    # DMA-transposes and each chunk's x^T loads as ONE DMA whose per-partition
    # data is a contiguous 6KB run (128 large descriptors, ~line rate).
    n_ich_d = t // ICH_W
    x_d = nc.dram_tensor("x", [128, n_ich_d, KT, ICH_W], BF16,
                         kind="ExternalInput").ap()
    # qk projection weights, 4 chunks of 128 output channels:
    # ch0=[q0|q1] ch1=[k0|k1] ch2=[q2|k2] ch3=[k2|q2]
    wqk_d = nc.dram_tensor("wqk", [D, 512], BF16, kind="ExternalInput").ap()
    bqk_d = nc.dram_tensor("bqk", [512], F32, kind="ExternalInput").ap()
    wv_d = nc.dram_tensor("wv", [D, HPC * DK], BF16, kind="ExternalInput").ap()
    wout_d = nc.dram_tensor("wout", [HPC * DK, D], BF16,
                            kind="ExternalInput").ap()
    out_d = nc.dram_tensor("out", [t, D], F32, kind="ExternalOutput").ap()

    with tile.TileContext(nc) as tc, ExitStack() as top:
        consts = top.enter_context(tc.tile_pool(name="consts", bufs=1))
        persist = top.enter_context(tc.tile_pool(name="persist", bufs=1))
        xtp = top.enter_context(tc.tile_pool(name="xt", bufs=2))
        ptp = top.enter_context(tc.tile_pool(name="pt", bufs=3))
        ctxp = top.enter_context(tc.tile_pool(name="ctx", bufs=12))
        smp = top.enter_context(tc.tile_pool(name="sm", bufs=4))
        outp = top.enter_context(tc.tile_pool(name="outsb", bufs=2))
        # PSUM: st 2 banks x2 + cps 1 bank x2 + pj 1 bank + op 1 bank = 8
        stp = top.enter_context(tc.tile_pool(name="stp", bufs=2, space="PSUM"))
        cpp = top.enter_context(tc.tile_pool(name="cpp", bufs=2, space="PSUM"))
        pjp = top.enter_context(tc.tile_pool(name="pjp", bufs=1, space="PSUM"))
        opp = top.enter_context(tc.tile_pool(name="opp", bufs=1, space="PSUM"))

        # q^T / k^T per chunk: [128, 4, t] bf16
        qk_sb = persist.tile([128, 4, t], BF16)
        # v (natural layout) + ones column: [128, n_tch, HPC, 65] bf16
        vaug = persist.tile([128, t // 128, HPC, DK + 1], BF16)

        xts = {}
        ctxn_store = {}

        def issue_xt(ich):
            """Prefetch chunk ich's x^T via xbar DMA-transpose.

            The first chunks split their transposes across BOTH HWDGE
            queues (sync + scalar): at startup the single sync queue
            serializes at ~2.7us per transpose (issue + transfer, ~2 in
            flight) which starves the first projections; the ACT queue is
            idle then.  Later chunks keep everything on sync so the
            descriptor generation (~1.3us each) never displaces exp work
            on the ACT sequencer."""
            if ich >= n_ich:
                return
            xt = xtp.tile([128, KT, ICH_W], BF16, tag="xt", name=f"xt{ich}")
            nc.sync.dma_start(out=xt, in_=x_d[:, ich])
            xts[ich] = xt

        IDENT = mybir.ActivationFunctionType.Identity

        def proj_quanta(ich):
            """Closures emitting chunk ich's projections.

            Each qk/v group is ONE quantum of 6 matmuls plus a separate
            epilogue quantum on the ACT engine (Identity with per-partition
            bias) so the pj PSUM ring slot is held for only one filler point
            and its release never queues behind long DVE ops."""
            i0 = ich * ICH_W
            qlist = []

            def mk_qk(ch):
                st_ = {}

                def q0():
                    ps = pjp.tile([128, ICH_W], F32, tag="pj", space="PSUM",
                                  name=f"qps{ich}_{ch}")
                    st_["ps"] = ps
                    for kt in range(KT):
                        nc.tensor.matmul(
                            ps, lhsT=wqk_sb[:, kt, ch * 128:(ch + 1) * 128],
                            rhs=xts[ich][:, kt, :],
                            start=(kt == 0), stop=(kt == KT - 1))

                def q1():
                    nc.scalar.activation(
                        qk_sb[:, ch, i0:i0 + ICH_W], st_["ps"], IDENT,
                        bias=bqk_sb[:, ch:ch + 1], scale=1.0)

                return [q0, q1]

            def mk_v(tl):
                st_ = {}
                tch = ich * (ICH_W // 128) + tl

                def q0():
                    ps = pjp.tile([128, ICH_W], F32, tag="pj", space="PSUM",
                                  name=f"vps{ich}_{tl}")
                    st_["ps"] = ps
                    for kt in range(KT):
                        nc.tensor.matmul(
                            ps[:, 0:HPC * DK],
                            lhsT=xts[ich][:, kt, tl * 128:(tl + 1) * 128],
                            rhs=wv_sb[:, kt, :],
                            start=(kt == 0), stop=(kt == KT - 1))

                def q1():
                    nc.scalar.activation(
                        vaug[:, tch, :, 0:DK],
                        st_["ps"][:, 0:HPC * DK].rearrange(
                            "p (h d) -> p h d", h=HPC),
                        IDENT, bias=0.0, scale=1.0)

                return [q0, q1]

            for ch in range(4):
                qlist += mk_qk(ch)
            for tl in range(ICH_W // 128):
                qlist += mk_v(tl)
            qlist.append(lambda: issue_xt(ich + 1))
            return qlist

        def outproj_quanta(ich):
            """Closures emitting chunk ich's out-projection in quanta."""
            i0 = ich * ICH_W
            qlist = []

            def mk(tsub):
                st_ = {}

                def q0():
                    # normalize muls for this chunk are deferred closures;
                    # make sure they have landed before reading ctxn.
                    while len(ctxn_store.get(ich, {})) < HPC:
                        deferred.pop(0)()
                    ctxn = ctxn_store[ich]
                    st_["osb"] = outp.tile([128, D], F32, tag="osb",
                                           name=f"osb{ich}_{tsub}")
                    o1 = opp.tile([128, 512], F32, tag="op", space="PSUM",
                                  name=f"op1_{ich}_{tsub}")
                    st_["o1"] = o1
                    for h in range(HPC):
                        nc.tensor.matmul(
                            o1, lhsT=ctxn[h][:, tsub * 128:(tsub + 1) * 128],
                            rhs=wout_sb[:, h, 0:512],
                            start=(h == 0), stop=(h == HPC - 1))

                def q1():
                    nc.vector.tensor_copy(st_["osb"][:, 0:512], st_["o1"])

                def q2():
                    ctxn = ctxn_store[ich]
                    o2 = opp.tile([128, 512], F32, tag="op", space="PSUM",
                                  name=f"op2_{ich}_{tsub}")
                    st_["o2"] = o2
                    for h in range(HPC):
                        nc.tensor.matmul(
                            o2[:, 0:256],
                            lhsT=ctxn[h][:, tsub * 128:(tsub + 1) * 128],
                            rhs=wout_sb[:, h, 512:D],
                            start=(h == 0), stop=(h == HPC - 1))

                def q3():
                    nc.vector.tensor_copy(st_["osb"][:, 512:D],
                                          st_["o2"][:, 0:256])
                    nc.sync.dma_start(
                        out=out_d[i0 + tsub * 128:i0 + (tsub + 1) * 128, :],
                        in_=st_["osb"])

                return [q0, q1, q2, q3]

            for tsub in range(ICH_W // 128):
                qlist += mk(tsub)
            return qlist

        deferred = []   # small DVE/gpsimd closures spread across filler points

        def normalize(ich, h, cps):
            # copy PSUM->SBUF immediately so the cps ring slot frees in
            # ~0.7us.  The division itself runs on GPSIMD (broadcast the raw
            # denominator, then tensor/tensor divide): a DVE reciprocal is
            # ~6.5 cyc/elem and its ~10us/chunk mass in the FIFO was stalling
            # the small epilogue ops that gate PE matmuls.
            ctxf = smp.tile([65, ICH_W], F32, tag="ctxf", bufs=8,
                            name=f"cf{ich}{h}")
            nc.vector.tensor_copy(ctxf, cps)
            recip = smp.tile([1, ICH_W], F32, tag="recip", bufs=8,
                             name=f"rc{ich}{h}")

            def mk_recip(c):
                return lambda: nc.vector.reciprocal(
                    recip[:, c * 128:(c + 1) * 128],
                    ctxf[64:65, c * 128:(c + 1) * 128])

            def bcast():
                rb = smp.tile([64, ICH_W], F32, tag="rb", bufs=6,
                              name=f"rb{ich}{h}")
                nc.gpsimd.partition_broadcast(rb, recip)
                st_["rb"] = rb

            def mul():
                cn = ctxp.tile([64, ICH_W], BF16, tag="ctxn",
                               name=f"cn{ich}{h}")
                nc.vector.tensor_mul(cn, ctxf[0:64, :], st_["rb"])
                ctxn_store.setdefault(ich, {})[h] = cn

            st_ = {}
            deferred.extend([mk_recip(c) for c in range(ICH_W // 128)])
            deferred.append(bcast)
            deferred.append(mul)

        def attn_chunk(ich, fill):
            i0 = ich * ICH_W
            njb = (i0 + ICH_W) // JB_W
            points = njb + njb // 2
            fk = [0]

            def filler():
                n = 1 + (len(deferred) > 6) + (len(deferred) > 12)
                for _ in range(min(n, len(deferred))):
                    deferred.pop(0)()
                a = fk[0] * len(fill) // points
                b = (fk[0] + 1) * len(fill) // points
                for f in fill[a:b]:
                    f()
                fk[0] += 1

            # ---- pass A: heads 0 and 1, row-group paired ----
            cps0 = cpp.tile([65, ICH_W], F32, tag="cps", space="PSUM",
                            name=f"cps0_{ich}")
            cps1 = cpp.tile([65, ICH_W], F32, tag="cps", space="PSUM",
                            name=f"cps1_{ich}")
            for jb in range(njb):
                j0 = jb * JB_W
                s = jb - (njb - 4)          # diag position if >= 0
                c0 = 128 * s if s > 0 else 0
                st = stp.tile([128, 2, ICH_W], F32, tag="st", space="PSUM",
                              name=f"stA{ich}_{jb}")
                nc.tensor.matmul(
                    st[:, 0, c0:],
                    lhsT=qk_sb[0:64, 1, j0:j0 + JB_W],
                    rhs=qk_sb[0:64, 0, i0 + c0:i0 + ICH_W],
                    start=True, stop=True)
                nc.tensor.matmul(
                    st[:, 1, c0:],
                    lhsT=qk_sb[64:128, 1, j0:j0 + JB_W],
                    rhs=qk_sb[64:128, 0, i0 + c0:i0 + ICH_W],
                    start=True, stop=True)
                pt = ptp.tile([128, 2, ICH_W], BF16, tag="pt",
                              name=f"ptA{ich}_{jb}")
                nc.scalar.activation(pt[:, :, c0:], st[:, :, c0:], EXP,
                                     bias=0.0, scale=SCALE)
                if s >= 0:
                    for hh in range(2):
                        nc.gpsimd.affine_select(
                            out=pt[:, hh, c0:c0 + 128],
                            in_=pt[:, hh, c0:c0 + 128],
                            compare_op=mybir.AluOpType.is_ge,
                            fill=0.0, base=0,
                            pattern=[[1, 128]], channel_multiplier=-1)
                filler()
                nc.tensor.matmul(
                    cps0[:, c0:], lhsT=vaug[:, jb, 0, :], rhs=pt[:, 0, c0:],
                    start=(jb == 0), stop=(jb == njb - 1))
                nc.tensor.matmul(
                    cps1[:, c0:], lhsT=vaug[:, jb, 1, :], rhs=pt[:, 1, c0:],
                    start=(jb == 0), stop=(jb == njb - 1))

            normalize(ich, 0, cps0)
            normalize(ich, 1, cps1)

            # ---- pass B: head 2, alternating row groups ----
            cps2 = cpp.tile([65, ICH_W], F32, tag="cps", space="PSUM",
                            name=f"cps2_{ich}")
            for grp in range(njb // 2):
                st = stp.tile([128, 2, ICH_W], F32, tag="st", space="PSUM",
                              name=f"stB{ich}_{grp}")
                pt = ptp.tile([128, 2, ICH_W], BF16, tag="pt",
                              name=f"ptB{ich}_{grp}")
                c0s = []
                for jj in range(2):
                    jb = grp * 2 + jj
                    j0 = jb * JB_W
                    s = jb - (njb - 4)
                    c0 = 128 * s if s > 0 else 0
                    c0s.append((s, c0))
                    if jb % 2 == 0:
                        lhsT = qk_sb[0:64, 3, j0:j0 + JB_W]
                        rhs = qk_sb[0:64, 2, i0 + c0:i0 + ICH_W]
                    else:
                        lhsT = qk_sb[64:128, 2, j0:j0 + JB_W]
                        rhs = qk_sb[64:128, 3, i0 + c0:i0 + ICH_W]
                    nc.tensor.matmul(st[:, jj, c0:], lhsT=lhsT, rhs=rhs,
                                     start=True, stop=True)
                if c0s[0][0] < 0 and c0s[1][0] < 0:
                    # off-diagonal pair: one fused exp
                    nc.scalar.activation(pt, st, EXP, bias=0.0, scale=SCALE)
                else:
                    for jj in range(2):
                        c0 = c0s[jj][1]
                        nc.scalar.activation(pt[:, jj, c0:], st[:, jj, c0:],
                                             EXP, bias=0.0, scale=SCALE)
                for jj in range(2):
                    s, c0 = c0s[jj]
                    if s >= 0:
                        nc.gpsimd.affine_select(
                            out=pt[:, jj, c0:c0 + 128],
                            in_=pt[:, jj, c0:c0 + 128],
                            compare_op=mybir.AluOpType.is_ge,
                            fill=0.0, base=0,
                            pattern=[[1, 128]], channel_multiplier=-1)
                filler()
                for jj in range(2):
                    jb = grp * 2 + jj
                    c0 = c0s[jj][1]
                    nc.tensor.matmul(
                        cps2[:, c0:], lhsT=vaug[:, jb, 2, :],
                        rhs=pt[:, jj, c0:],
                        start=(jb == 0), stop=(jb == njb - 1))

            normalize(ich, 2, cps2)
            # drain any leftover filler (rounding)
            for f in fill[fk[0] * len(fill) // points:]:
                f()

        # ---------------- main schedule ----------------
        # Startup: interleave weight loads with chunk 0's x^T transposes so
        # the first projection matmuls start as soon as their operands land.
        wqk_r = wqk_d.rearrange("(kt p) c -> p kt c", p=128)
        wqk_sb = consts.tile([128, KT, 512], BF16)
        bqk_sb = consts.tile([128, 4], F32)
        wv_sb = consts.tile([128, KT, HPC * DK], BF16)
        wout_sb = consts.tile([64, HPC, D], BF16)
        nc.sync.dma_start(out=wqk_sb[:, 0:2, :], in_=wqk_r[:, 0:2, :])
        nc.sync.dma_start(out=bqk_sb,
                          in_=bqk_d.rearrange("(ch p) -> p ch", p=128))
        issue_xt(0)
        xt0 = xts[0]
        nc.sync.dma_start(out=wqk_sb[:, 2:KT, :], in_=wqk_r[:, 2:KT, :])
        nc.sync.dma_start(out=wv_sb,
                          in_=wv_d.rearrange("(kt p) c -> p kt c", p=128))
        nc.sync.dma_start(out=wout_sb,
                          in_=wout_d.rearrange("(h p) m -> p h m", p=64))
        # softmax-denominator ones column, written once
        nc.vector.memset(vaug[:, :, :, DK:DK + 1], 1.0)
        # proj(0) runs kt-major over PAIRS of output chunks (using the idle
        # pj+op rings) so each matmul waits only on its own x^T transpose
        # instead of the whole 6-DMA chain.
        for ch0 in (0, 2):
            psa = pjp.tile([128, ICH_W], F32, tag="pj", space="PSUM",
                           name=f"qps0_{ch0}")
            psb = opp.tile([128, ICH_W], F32, tag="op", space="PSUM",
                           name=f"qps0_{ch0 + 1}")
            for kt in range(KT):
                for ps, ch in ((psa, ch0), (psb, ch0 + 1)):
                    nc.tensor.matmul(
                        ps, lhsT=wqk_sb[:, kt, ch * 128:(ch + 1) * 128],
                        rhs=xt0[:, kt, :],
                        start=(kt == 0), stop=(kt == KT - 1))
            for ps, ch in ((psa, ch0), (psb, ch0 + 1)):
                nc.scalar.activation(
                    qk_sb[:, ch, 0:ICH_W], ps,
                    mybir.ActivationFunctionType.Identity,
                    bias=bqk_sb[:, ch:ch + 1], scale=1.0)
        for tl in range(ICH_W // 128):
            ps = pjp.tile([128, ICH_W], F32, tag="pj", space="PSUM",
                          name=f"vps0_{tl}")
            for kt in range(KT):
                nc.tensor.matmul(
                    ps[:, 0:HPC * DK],
                    lhsT=xt0[:, kt, tl * 128:(tl + 1) * 128],
                    rhs=wv_sb[:, kt, :],
                    start=(kt == 0), stop=(kt == KT - 1))
            nc.scalar.activation(
                vaug[:, tl, :, 0:DK],
                ps[:, 0:HPC * DK].rearrange("p (h d) -> p h d", h=HPC),
                mybir.ActivationFunctionType.Identity, bias=0.0, scale=1.0)
        issue_xt(1)
        # outproj(ich) is hosted three chunks later: the ctxn normalize chain
        # gets two full chunks of slack (its deferred DVE ops never gate the
        # outproj matmuls) and the last, ACT-bound chunk receives ~20us of
        # real PE filler, which also keeps the HAM clock warm there.
        hosts = {src: min(src + 3, n_ich - 1) for src in range(n_ich - 1)}
        for ich in range(n_ich):
            fill = []
            if ich + 1 < n_ich:
                fill += proj_quanta(ich + 1)
            for src, host in hosts.items():
                if host == ich:
                    fill += outproj_quanta(src)
            attn_chunk(ich, fill)
        while deferred:
            deferred.pop(0)()
        for q in outproj_quanta(n_ich - 1):
            q()

    nc.compile()
    return nc


def make_core_inputs(x_b, W_qkv, b_qkv, W_out, hg):
    """Host-side weight slicing/permutation for one head-group hg (0..3)."""
    heads = [hg * HPC + i for i in range(HPC)]
    # W_qkv last-dim layout: c = h*192 + s*64 + d  (s: 0=q 1=k 2=v)
    def cols(h, s):
        return slice(h * 192 + s * 64, h * 192 + s * 64 + 64)

    q = [np.asarray(W_qkv[:, cols(h, 0)]) for h in heads]
    k = [np.asarray(W_qkv[:, cols(h, 1)]) for h in heads]
    v = [np.asarray(W_qkv[:, cols(h, 2)]) for h in heads]
    bq = [np.asarray(b_qkv[cols(h, 0)]) for h in heads]

    wqk = np.concatenate([q[0], q[1], k[0], k[1], q[2], k[2], k[2], q[2]],
                         axis=1)
    z = np.zeros(64, np.float32)
    bqk = np.concatenate([bq[0], bq[1], z, z, bq[2], z, z, bq[2]]).astype(
        np.float32)
    wv = np.concatenate(v, axis=1)
    wout = np.concatenate(
        [np.asarray(W_out[h * DK:(h + 1) * DK, :]) for h in heads], axis=0)
    # [t, D] -> [p, ich, kt, tl]: x^T chunk-major (see build_program)
    t_loc = np.asarray(x_b).shape[0]
    xprep = np.asarray(x_b).astype(NPBF16).T.reshape(
        KT, 128, t_loc // ICH_W, ICH_W).transpose(1, 2, 0, 3)
    return {
        "x": np.ascontiguousarray(xprep),
        "wqk": np.ascontiguousarray(wqk.astype(NPBF16)),
        "bqk": np.ascontiguousarray(bqk),
        "wv": np.ascontiguousarray(wv.astype(NPBF16)),
        "wout": np.ascontiguousarray(wout.astype(NPBF16)),
    }


_CACHE = {}


def _get_program(t=T):
    if t not in _CACHE:
        _CACHE[t] = build_program(t)
    return _CACHE[t]


def run_cores(inputs, t=T, trace=False):
    nc = _get_program(t)
    x = np.asarray(inputs["x"], np.float32)
    in_maps = []
    for core in range(N_CORES):
        b, hg = core // 4, core % 4
        in_maps.append(make_core_inputs(x[b], inputs["W_qkv"],
                                        inputs["b_qkv"], inputs["W_out"], hg))
    res = run_bass_kernel_spmd(nc, in_maps, list(range(N_CORES)), trace=trace)
    return res


def gather(inputs, results):
    b_qkv = np.asarray(inputs["b_qkv"], np.float32)
    W_out = np.asarray(inputs["W_out"], np.float32)
    b_out = np.asarray(inputs["b_out"], np.float32)
    bv = np.concatenate([b_qkv[h * 192 + 128:h * 192 + 192] for h in range(H)])
    fold = bv @ W_out + b_out                      # [D]
    t = results[0]["out"].shape[0]
    out = np.zeros((B, t, D), np.float32)
    for core in range(N_CORES):
        out[core // 4] += results[core]["out"]
    out += fold[None, None, :]
    return out


def kernel(**inputs):
    res = run_cores(inputs)
    return gather(inputs, res.results)


if __name__ == "__main__":
    # smoke test with random data
    rng = np.random.default_rng(0)
    inputs = {
        "x": rng.standard_normal((B, T, D), dtype=np.float32),
        "mask": np.triu(np.ones((T, T), dtype=bool), k=1),
        "W_qkv": (rng.standard_normal((D, 3 * D), dtype=np.float32)
                  / np.sqrt(D)),
        "b_qkv": rng.standard_normal(3 * D).astype(np.float32) * 0.02,
        "W_out": (rng.standard_normal((D, D), dtype=np.float32)
                  / np.sqrt(D)),
        "b_out": rng.standard_normal(D).astype(np.float32) * 0.02,
    }
    out = kernel(**inputs)
    print(out.shape, out.dtype)


# revision 38
# speedup vs baseline: 1.2077x; 1.0125x over previous
"""Trainium2 Bass kernel for causal multi-head attention (v3, interleaved).

Problem: B=2, T=4096, D=768, H=12 heads, d_k=64, causal mask.
Sharding: 8 cores = 2 batches x 4 head-groups (3 heads each).
Each core computes its batch's qkv projection (its heads only), flash-style
attention with transposed scores (S^T = k q^T, so softmax statistics land in
the matmul-friendly layout with no P-transposes), and a partial output
projection. Host sums the 4 head-group partials per batch and adds the
folded bias constant (v-bias @ W_out + b_out). The k-bias is dropped
entirely (softmax is invariant to per-query score shifts).

v3: the qkv-projection and out-projection matmuls are interleaved into the
attention j-block loop as PE "filler" quanta.  In v2 the attention pipeline
alternated PE<->ACT with ~200-600ns PE idle per j-block, which kept the PE
HAM throttle oscillating (338us at half clock).  Filler keeps the PE stream
dense: chunk ich's attention hosts proj(ich+1) and outproj(ich-1) quanta,
placed between the score matmuls/exp and the context matmuls of each
j-block, exactly where the PE would otherwise wait for the exp.

All matmul operands bf16 (host-cast; tolerance is 2e-2, bf16 noise ~3e-3).
x^T comes from xbar DMA-transpose.  exp/score/context skip fully-masked
query columns of diagonal j-blocks.

Self-contained: hardcodes all shapes; only imports the concourse runtime.
"""

import sys

sys.path.insert(0, "/opt/trn_rl_repo")

from contextlib import ExitStack

import ml_dtypes
import numpy as np

import concourse.bass as bass  # noqa: F401
import concourse.mybir as mybir
import concourse.tile as tile
from concourse import bacc
from concourse.bass_utils import run_bass_kernel_spmd

F32 = mybir.dt.float32
BF16 = mybir.dt.bfloat16
NPBF16 = ml_dtypes.bfloat16

B, T, D = 2, 4096, 768
H, DK = 12, 64
HPC = 3          # heads per core
N_CORES = 8
ICH_W = 512      # i-chunk width (queries per outer step)
JB_W = 128       # j-block width (keys per matmul)
KT = D // 128    # 6 contraction tiles for the projections
EXP = mybir.ActivationFunctionType.Exp
SCALE = 1.0 / np.sqrt(DK)


def build_program(t=T):
    """Build the SPMD Bass program for one core (all cores identical)."""
    n_ich = t // ICH_W

    nc = bacc.Bacc("TRN2", target_bir_lowering=False, debug=False,
                   num_devices=N_CORES)

    # x arrives pre-transposed AND chunk-major from the host:
    # x[p, ich, kt, tl] = x^T[kt*128+p, ich*512+tl].  This removes all xbar
    # DMA-transposes and each chunk's x^T loads as ONE DMA whose per-partition
    # data is a contiguous 6KB run (128 large descriptors, ~line rate).
    n_ich_d = t // ICH_W
    x_d = nc.dram_tensor("x", [128, n_ich_d, KT, ICH_W], BF16,
                         kind="ExternalInput").ap()
    # qk projection weights, 4 chunks of 128 output channels:
    # ch0=[q0|q1] ch1=[k0|k1] ch2=[q2|k2] ch3=[k2|q2]
    wqk_d = nc.dram_tensor("wqk", [D, 512], BF16, kind="ExternalInput").ap()
    bqk_d = nc.dram_tensor("bqk", [512], F32, kind="ExternalInput").ap()
    wv_d = nc.dram_tensor("wv", [D, HPC * DK], BF16, kind="ExternalInput").ap()
    wout_d = nc.dram_tensor("wout", [HPC * DK, D], BF16,
                            kind="ExternalInput").ap()
    out_d = nc.dram_tensor("out", [t, D], F32, kind="ExternalOutput").ap()

    with tile.TileContext(nc) as tc, ExitStack() as top:
        consts = top.enter_context(tc.tile_pool(name="consts", bufs=1))
        persist = top.enter_context(tc.tile_pool(name="persist", bufs=1))
        xtp = top.enter_context(tc.tile_pool(name="xt", bufs=2))
        ptp = top.enter_context(tc.tile_pool(name="pt", bufs=3))
        ctxp = top.enter_context(tc.tile_pool(name="ctx", bufs=12))
        smp = top.enter_context(tc.tile_pool(name="sm", bufs=4))
        outp = top.enter_context(tc.tile_pool(name="outsb", bufs=2))
        # PSUM: st 2 banks x2 + cps 1 bank x2 + pj 1 bank + op 1 bank = 8
        stp = top.enter_context(tc.tile_pool(name="stp", bufs=2, space="PSUM"))
        cpp = top.enter_context(tc.tile_pool(name="cpp", bufs=2, space="PSUM"))
        pjp = top.enter_context(tc.tile_pool(name="pjp", bufs=1, space="PSUM"))
        opp = top.enter_context(tc.tile_pool(name="opp", bufs=1, space="PSUM"))

        # q^T / k^T per chunk: [128, 4, t] bf16
        qk_sb = persist.tile([128, 4, t], BF16)
        # v (natural layout) + ones column: [128, n_tch, HPC, 65] bf16
        vaug = persist.tile([128, t // 128, HPC, DK + 1], BF16)

        xts = {}
        ctxn_store = {}

        def issue_xt(ich):
            """Prefetch chunk ich's x^T via xbar DMA-transpose.

            The first chunks split their transposes across BOTH HWDGE
            queues (sync + scalar): at startup the single sync queue
            serializes at ~2.7us per transpose (issue + transfer, ~2 in
            flight) which starves the first projections; the ACT queue is
            idle then.  Later chunks keep everything on sync so the
            descriptor generation (~1.3us each) never displaces exp work
            on the ACT sequencer."""
            if ich >= n_ich:
                return
            xt = xtp.tile([128, KT, ICH_W], BF16, tag="xt", name=f"xt{ich}")
            nc.sync.dma_start(out=xt, in_=x_d[:, ich])
            xts[ich] = xt

        IDENT = mybir.ActivationFunctionType.Identity

        def proj_quanta(ich):
            """Closures emitting chunk ich's projections.

            Each qk/v group is ONE quantum of 6 matmuls plus a separate
            epilogue quantum on the ACT engine (Identity with per-partition
            bias) so the pj PSUM ring slot is held for only one filler point
            and its release never queues behind long DVE ops."""
            i0 = ich * ICH_W
            qlist = []

            def mk_qk(ch):
                st_ = {}

                def q0():
                    ps = pjp.tile([128, ICH_W], F32, tag="pj", space="PSUM",
                                  name=f"qps{ich}_{ch}")
                    st_["ps"] = ps
                    for kt in range(KT):
                        nc.tensor.matmul(
                            ps, lhsT=wqk_sb[:, kt, ch * 128:(ch + 1) * 128],
                            rhs=xts[ich][:, kt, :],
                            start=(kt == 0), stop=(kt == KT - 1))

                def q1():
                    nc.scalar.activation(
                        qk_sb[:, ch, i0:i0 + ICH_W], st_["ps"], IDENT,
                        bias=bqk_sb[:, ch:ch + 1], scale=1.0)

                return [q0, q1]

            def mk_v(tl):
                st_ = {}
                tch = ich * (ICH_W // 128) + tl

                def q0():
                    ps = pjp.tile([128, ICH_W], F32, tag="pj", space="PSUM",
                                  name=f"vps{ich}_{tl}")
                    st_["ps"] = ps
                    for kt in range(KT):
                        nc.tensor.matmul(
                            ps[:, 0:HPC * DK],
                            lhsT=xts[ich][:, kt, tl * 128:(tl + 1) * 128],
                            rhs=wv_sb[:, kt, :],
                            start=(kt == 0), stop=(kt == KT - 1))

                def q1():
                    nc.scalar.activation(
                        vaug[:, tch, :, 0:DK],
                        st_["ps"][:, 0:HPC * DK].rearrange(
                            "p (h d) -> p h d", h=HPC),
                        IDENT, bias=0.0, scale=1.0)

                return [q0, q1]

            for ch in range(4):
                qlist += mk_qk(ch)
            for tl in range(ICH_W // 128):
                qlist += mk_v(tl)
            qlist.append(lambda: issue_xt(ich + 1))
            return qlist

        def outproj_quanta(ich):
            """Closures emitting chunk ich's out-projection in quanta."""
            i0 = ich * ICH_W
            qlist = []

            def mk(tsub):
                st_ = {}

                def q0():
                    # normalize muls for this chunk are deferred closures;
                    # make sure they have landed before reading ctxn.
                    while len(ctxn_store.get(ich, {})) < HPC:
                        deferred.pop(0)()
                    ctxn = ctxn_store[ich]
                    st_["osb"] = outp.tile([128, D], F32, tag="osb",
                                           name=f"osb{ich}_{tsub}")
                    o1 = opp.tile([128, 512], F32, tag="op", space="PSUM",
                                  name=f"op1_{ich}_{tsub}")
                    st_["o1"] = o1
                    for h in range(HPC):
                        nc.tensor.matmul(
                            o1, lhsT=ctxn[h][:, tsub * 128:(tsub + 1) * 128],
                            rhs=wout_sb[:, h, 0:512],
                            start=(h == 0), stop=(h == HPC - 1))

                def q1():
                    nc.vector.tensor_copy(st_["osb"][:, 0:512], st_["o1"])

                def q2():
                    ctxn = ctxn_store[ich]
                    o2 = opp.tile([128, 512], F32, tag="op", space="PSUM",
                                  name=f"op2_{ich}_{tsub}")
                    st_["o2"] = o2
                    for h in range(HPC):
                        nc.tensor.matmul(
                            o2[:, 0:256],
                            lhsT=ctxn[h][:, tsub * 128:(tsub + 1) * 128],
                            rhs=wout_sb[:, h, 512:D],
                            start=(h == 0), stop=(h == HPC - 1))

                def q3():
                    nc.vector.tensor_copy(st_["osb"][:, 512:D],
                                          st_["o2"][:, 0:256])
                    nc.sync.dma_start(
                        out=out_d[i0 + tsub * 128:i0 + (tsub + 1) * 128, :],
                        in_=st_["osb"])

                return [q0, q1, q2, q3]

            for tsub in range(ICH_W // 128):
                qlist += mk(tsub)
            return qlist

        deferred = []   # small DVE/gpsimd closures spread across filler points

        def normalize(ich, h, cps):
            # copy PSUM->SBUF immediately so the cps ring slot frees in
            # ~0.7us.  The division itself runs on GPSIMD (broadcast the raw
            # denominator, then tensor/tensor divide): a DVE reciprocal is
            # ~6.5 cyc/elem and its ~10us/chunk mass in the FIFO was stalling
            # the small epilogue ops that gate PE matmuls.
            ctxf = smp.tile([65, ICH_W], F32, tag="ctxf", bufs=8,
                            name=f"cf{ich}{h}")
            nc.vector.tensor_copy(ctxf, cps)
            recip = smp.tile([1, ICH_W], F32, tag="recip", bufs=8,
                             name=f"rc{ich}{h}")

            def mk_recip(c):
                return lambda: nc.vector.reciprocal(
                    recip[:, c * 64:(c + 1) * 64],
                    ctxf[64:65, c * 64:(c + 1) * 64])

            def bcast():
                rb = smp.tile([64, ICH_W], F32, tag="rb", bufs=6,
                              name=f"rb{ich}{h}")
                nc.gpsimd.partition_broadcast(rb, recip)
                st_["rb"] = rb

            def mul():
                cn = ctxp.tile([64, ICH_W], BF16, tag="ctxn",
                               name=f"cn{ich}{h}")
                nc.vector.tensor_mul(cn, ctxf[0:64, :], st_["rb"])
                ctxn_store.setdefault(ich, {})[h] = cn

            st_ = {}
            deferred.extend([mk_recip(c) for c in range(ICH_W // 64)])
            deferred.append(bcast)
            deferred.append(mul)

        def attn_chunk(ich, fill):
            i0 = ich * ICH_W
            njb = (i0 + ICH_W) // JB_W
            points = njb + njb // 2
            fk = [0]

            def filler():
                # strictly ONE deferred DVE op per point: popping more
                # bunches reciprocals back-to-back in the DVE FIFO, which
                # then stalls the PE-gating epilogue/copy ops behind them.
                if deferred:
                    deferred.pop(0)()
                a = fk[0] * len(fill) // points
                b = (fk[0] + 1) * len(fill) // points
                for f in fill[a:b]:
                    f()
                fk[0] += 1

            # ---- pass A: heads 0 and 1, row-group paired ----
            cps0 = cpp.tile([65, ICH_W], F32, tag="cps", space="PSUM",
                            name=f"cps0_{ich}")
            cps1 = cpp.tile([65, ICH_W], F32, tag="cps", space="PSUM",
                            name=f"cps1_{ich}")
            for jb in range(njb):
                j0 = jb * JB_W
                s = jb - (njb - 4)          # diag position if >= 0
                c0 = 128 * s if s > 0 else 0
                st = stp.tile([128, 2, ICH_W], F32, tag="st", space="PSUM",
                              name=f"stA{ich}_{jb}")
                nc.tensor.matmul(
                    st[:, 0, c0:],
                    lhsT=qk_sb[0:64, 1, j0:j0 + JB_W],
                    rhs=qk_sb[0:64, 0, i0 + c0:i0 + ICH_W],
                    start=True, stop=True)
                nc.tensor.matmul(
                    st[:, 1, c0:],
                    lhsT=qk_sb[64:128, 1, j0:j0 + JB_W],
                    rhs=qk_sb[64:128, 0, i0 + c0:i0 + ICH_W],
                    start=True, stop=True)
                pt = ptp.tile([128, 2, ICH_W], BF16, tag="pt",
                              name=f"ptA{ich}_{jb}")
                nc.scalar.activation(pt[:, :, c0:], st[:, :, c0:], EXP,
                                     bias=0.0, scale=SCALE)
                if s >= 0:
                    for hh in range(2):
                        nc.gpsimd.affine_select(
                            out=pt[:, hh, c0:c0 + 128],
                            in_=pt[:, hh, c0:c0 + 128],
                            compare_op=mybir.AluOpType.is_ge,
                            fill=0.0, base=0,
                            pattern=[[1, 128]], channel_multiplier=-1)
                filler()
                nc.tensor.matmul(
                    cps0[:, c0:], lhsT=vaug[:, jb, 0, :], rhs=pt[:, 0, c0:],
                    start=(jb == 0), stop=(jb == njb - 1))
                nc.tensor.matmul(
                    cps1[:, c0:], lhsT=vaug[:, jb, 1, :], rhs=pt[:, 1, c0:],
                    start=(jb == 0), stop=(jb == njb - 1))

            normalize(ich, 0, cps0)
            normalize(ich, 1, cps1)

            # ---- pass B: head 2, alternating row groups ----
            cps2 = cpp.tile([65, ICH_W], F32, tag="cps", space="PSUM",
                            name=f"cps2_{ich}")
            for grp in range(njb // 2):
                st = stp.tile([128, 2, ICH_W], F32, tag="st", space="PSUM",
                              name=f"stB{ich}_{grp}")
                pt = ptp.tile([128, 2, ICH_W], BF16, tag="pt",
                              name=f"ptB{ich}_{grp}")
                c0s = []
                for jj in range(2):
                    jb = grp * 2 + jj
                    j0 = jb * JB_W
                    s = jb - (njb - 4)
                    c0 = 128 * s if s > 0 else 0
                    c0s.append((s, c0))
                    if jb % 2 == 0:
                        lhsT = qk_sb[0:64, 3, j0:j0 + JB_W]
                        rhs = qk_sb[0:64, 2, i0 + c0:i0 + ICH_W]
                    else:
                        lhsT = qk_sb[64:128, 2, j0:j0 + JB_W]
                        rhs = qk_sb[64:128, 3, i0 + c0:i0 + ICH_W]
                    nc.tensor.matmul(st[:, jj, c0:], lhsT=lhsT, rhs=rhs,
                                     start=True, stop=True)
                if c0s[0][0] < 0 and c0s[1][0] < 0:
                    # off-diagonal pair: one fused exp
                    nc.scalar.activation(pt, st, EXP, bias=0.0, scale=SCALE)
                else:
                    for jj in range(2):
                        c0 = c0s[jj][1]
                        nc.scalar.activation(pt[:, jj, c0:], st[:, jj, c0:],
                                             EXP, bias=0.0, scale=SCALE)
                for jj in range(2):
                    s, c0 = c0s[jj]
                    if s >= 0:
                        nc.gpsimd.affine_select(
                            out=pt[:, jj, c0:c0 + 128],
                            in_=pt[:, jj, c0:c0 + 128],
                            compare_op=mybir.AluOpType.is_ge,
                            fill=0.0, base=0,
                            pattern=[[1, 128]], channel_multiplier=-1)
                filler()
                for jj in range(2):
                    jb = grp * 2 + jj
                    c0 = c0s[jj][1]
                    nc.tensor.matmul(
                        cps2[:, c0:], lhsT=vaug[:, jb, 2, :],
                        rhs=pt[:, jj, c0:],
                        start=(jb == 0), stop=(jb == njb - 1))

            normalize(ich, 2, cps2)
            # drain any leftover filler (rounding)
            for f in fill[fk[0] * len(fill) // points:]:
                f()

        # ---------------- main schedule ----------------
        # Startup: interleave weight loads with chunk 0's x^T transposes so
        # the first projection matmuls start as soon as their operands land.
        wqk_r = wqk_d.rearrange("(kt p) c -> p kt c", p=128)
        wqk_sb = consts.tile([128, KT, 512], BF16)
        bqk_sb = consts.tile([128, 4], F32)
        wv_sb = consts.tile([128, KT, HPC * DK], BF16)
        wout_sb = consts.tile([64, HPC, D], BF16)
        nc.sync.dma_start(out=wqk_sb[:, 0:2, :], in_=wqk_r[:, 0:2, :])
        nc.sync.dma_start(out=bqk_sb,
                          in_=bqk_d.rearrange("(ch p) -> p ch", p=128))
        issue_xt(0)
        xt0 = xts[0]
        nc.sync.dma_start(out=wqk_sb[:, 2:KT, :], in_=wqk_r[:, 2:KT, :])
        nc.sync.dma_start(out=wv_sb,
                          in_=wv_d.rearrange("(kt p) c -> p kt c", p=128))
        nc.sync.dma_start(out=wout_sb,
                          in_=wout_d.rearrange("(h p) m -> p h m", p=64))
        # softmax-denominator ones column, written once
        nc.vector.memset(vaug[:, :, :, DK:DK + 1], 1.0)
        # proj(0) runs kt-major over PAIRS of output chunks (using the idle
        # pj+op rings) so each matmul waits only on its own x^T transpose
        # instead of the whole 6-DMA chain.
        for ch0 in (0, 2):
            psa = pjp.tile([128, ICH_W], F32, tag="pj", space="PSUM",
                           name=f"qps0_{ch0}")
            psb = opp.tile([128, ICH_W], F32, tag="op", space="PSUM",
                           name=f"qps0_{ch0 + 1}")
            for kt in range(KT):
                for ps, ch in ((psa, ch0), (psb, ch0 + 1)):
                    nc.tensor.matmul(
                        ps, lhsT=wqk_sb[:, kt, ch * 128:(ch + 1) * 128],
                        rhs=xt0[:, kt, :],
                        start=(kt == 0), stop=(kt == KT - 1))
            for ps, ch in ((psa, ch0), (psb, ch0 + 1)):
                nc.scalar.activation(
                    qk_sb[:, ch, 0:ICH_W], ps,
                    mybir.ActivationFunctionType.Identity,
                    bias=bqk_sb[:, ch:ch + 1], scale=1.0)
        for tl in range(ICH_W // 128):
            ps = pjp.tile([128, ICH_W], F32, tag="pj", space="PSUM",
                          name=f"vps0_{tl}")
            for kt in range(KT):
                nc.tensor.matmul(
                    ps[:, 0:HPC * DK],
                    lhsT=xt0[:, kt, tl * 128:(tl + 1) * 128],
                    rhs=wv_sb[:, kt, :],
                    start=(kt == 0), stop=(kt == KT - 1))
            nc.scalar.activation(
                vaug[:, tl, :, 0:DK],
                ps[:, 0:HPC * DK].rearrange("p (h d) -> p h d", h=HPC),
                mybir.ActivationFunctionType.Identity, bias=0.0, scale=1.0)
        issue_xt(1)
        # outproj(ich) is hosted three chunks later: the ctxn normalize chain
        # gets two full chunks of slack (its deferred DVE ops never gate the
        # outproj matmuls) and the last, ACT-bound chunk receives ~20us of
        # real PE filler, which also keeps the HAM clock warm there.
        hosts = {src: min(src + 3, n_ich - 1) for src in range(n_ich - 1)}
        for ich in range(n_ich):
            fill = []
            if ich + 1 < n_ich:
                fill += proj_quanta(ich + 1)
            for src, host in hosts.items():
                if host == ich:
                    fill += outproj_quanta(src)
            attn_chunk(ich, fill)
        # Final drain: interleave dummy matmuls (pj ring is free now) with
        # the deferred normalize ops so the PE HAM clock stays warm through
        # the tail; otherwise outproj(7) runs at half clock.
        def dummy_mm():
            dps = pjp.tile([128, ICH_W], F32, tag="pj", space="PSUM",
                           name="dwarm")
            nc.tensor.matmul(dps, lhsT=qk_sb[0:64, 0, 0:JB_W],
                             rhs=qk_sb[0:64, 0, 0:ICH_W],
                             start=True, stop=True)

        while deferred:
            dummy_mm()
            deferred.pop(0)()
        for q in outproj_quanta(n_ich - 1):
            dummy_mm()
            q()

    nc.compile()
    return nc


def make_core_inputs(x_b, W_qkv, b_qkv, W_out, hg):
    """Host-side weight slicing/permutation for one head-group hg (0..3)."""
    heads = [hg * HPC + i for i in range(HPC)]
    # W_qkv last-dim layout: c = h*192 + s*64 + d  (s: 0=q 1=k 2=v)
    def cols(h, s):
        return slice(h * 192 + s * 64, h * 192 + s * 64 + 64)

    q = [np.asarray(W_qkv[:, cols(h, 0)]) for h in heads]
    k = [np.asarray(W_qkv[:, cols(h, 1)]) for h in heads]
    v = [np.asarray(W_qkv[:, cols(h, 2)]) for h in heads]
    bq = [np.asarray(b_qkv[cols(h, 0)]) for h in heads]

    wqk = np.concatenate([q[0], q[1], k[0], k[1], q[2], k[2], k[2], q[2]],
                         axis=1)
    z = np.zeros(64, np.float32)
    bqk = np.concatenate([bq[0], bq[1], z, z, bq[2], z, z, bq[2]]).astype(
        np.float32)
    wv = np.concatenate(v, axis=1)
    wout = np.concatenate(
        [np.asarray(W_out[h * DK:(h + 1) * DK, :]) for h in heads], axis=0)
    # [t, D] -> [p, ich, kt, tl]: x^T chunk-major (see build_program)
    t_loc = np.asarray(x_b).shape[0]
    xprep = np.asarray(x_b).astype(NPBF16).T.reshape(
        KT, 128, t_loc // ICH_W, ICH_W).transpose(1, 2, 0, 3)
    return {
        "x": np.ascontiguousarray(xprep),
        "wqk": np.ascontiguousarray(wqk.astype(NPBF16)),
        "bqk": np.ascontiguousarray(bqk),
        "wv": np.ascontiguousarray(wv.astype(NPBF16)),
        "wout": np.ascontiguousarray(wout.astype(NPBF16)),
    }


_CACHE = {}


def _get_program(t=T):
    if t not in _CACHE:
        _CACHE[t] = build_program(t)
    return _CACHE[t]


def run_cores(inputs, t=T, trace=False):
    nc = _get_program(t)
    x = np.asarray(inputs["x"], np.float32)
    in_maps = []
    for core in range(N_CORES):
        b, hg = core // 4, core % 4
        in_maps.append(make_core_inputs(x[b], inputs["W_qkv"],
                                        inputs["b_qkv"], inputs["W_out"], hg))
    res = run_bass_kernel_spmd(nc, in_maps, list(range(N_CORES)), trace=trace)
    return res


def gather(inputs, results):
    b_qkv = np.asarray(inputs["b_qkv"], np.float32)
    W_out = np.asarray(inputs["W_out"], np.float32)
    b_out = np.asarray(inputs["b_out"], np.float32)
    bv = np.concatenate([b_qkv[h * 192 + 128:h * 192 + 192] for h in range(H)])
    fold = bv @ W_out + b_out                      # [D]
    t = results[0]["out"].shape[0]
    out = np.zeros((B, t, D), np.float32)
    for core in range(N_CORES):
        out[core // 4] += results[core]["out"]
    out += fold[None, None, :]
    return out


def kernel(**inputs):
    res = run_cores(inputs)
    return gather(inputs, res.results)


if __name__ == "__main__":
    # smoke test with random data
    rng = np.random.default_rng(0)
    inputs = {
        "x": rng.standard_normal((B, T, D), dtype=np.float32),
        "mask": np.triu(np.ones((T, T), dtype=bool), k=1),
        "W_qkv": (rng.standard_normal((D, 3 * D), dtype=np.float32)
                  / np.sqrt(D)),
        "b_qkv": rng.standard_normal(3 * D).astype(np.float32) * 0.02,
        "W_out": (rng.standard_normal((D, D), dtype=np.float32)
                  / np.sqrt(D)),
        "b_out": rng.standard_normal(D).astype(np.float32) * 0.02,
    }
    out = kernel(**inputs)
    print(out.shape, out.dtype)
